# revision 9
# baseline (speedup 1.0000x reference)
"""Trainium2 Bass kernel for nn_Attn (additive/Bahdanau-style attention).

Math (per batch b):
    Wh, We   = W[:, :D], W[:, D:]                       # [D,D] each
    energy   = tanh(enc @ We.T + hidden @ Wh.T + b)     # [S, D]
    scores   = energy @ v, masked to length, softmax    # [S]
    context  = scores @ enc                             # [D]

Sharding: data-parallel over batch B=16 across 8 cores (2 batches/core);
W, b, v replicated.

Numerics / layout (validated offline against the reference inputs:
worst-batch rel_err ~7.8e-3 vs the 2e-2 gate):
  - Pass-1 enc_proj runs mostly in fp8 e4m3 with MatmulPerfMode.DoubleRow
    (2 fp8 K-values per PE cell -> 2x bf16 throughput).  Quantization
    noise on the scores (~0.03 absolute) fails short-`lengths` batches
    whose softmax support is too small to average it out, so s < 512 is
    computed in bf16 instead: short batches become fully bf16-accurate
    and long batches keep enough support for the fp8 noise to wash out.
  - Both We variants are pre-scaled by 32 host-side (fp8: clears the
    e4m3 subnormal range; bf16: scale-free) so one 1/32 descale in the
    tanh activation covers every pass-1 tile.  fp8 K = 1024 maps to 4
    chunks of 256 with d = kc*256 + p*2 + i (pair axis = dim 1).
  - energy^T tiles are [e=128, s=1024] (two single-bank PSUM matmul
    groups per tile) so each tanh is one big ACT op with the per-batch
    bias (hid_proj + b) as a per-partition scalar.
  - hid_proj is computed transposed ([e-part, batch], bf16) as one
    64-matmul PSUM accumulation group -> bias needs no DRAM bounce.
  - The v-dot runs on the DVE as an all-bf16 scalar_tensor_tensor chain
    (2-byte SBUF operands unlock the DVE fast modes); the 128->1
    partition reduce is one 8-matmul PSUM group per [128, 1024] acc tile.
  - Scores live in [128, S/128] layout; masked softmax uses the static
    bound M = sum|v| >= max(score) instead of a max-reduce; the exp
    table is accurate even at exp(-25) (verified on hw); normalization
    by 1/sum is applied to the final context vector.
  - Pass 2 (context) re-loads enc in natural [s, d] layout as bf16, all
    tiles DMA'd ahead, and runs accumulating matmuls (attn column
    stationary, enc moving).
  - HBM/core: enc fp8 3.1MB + bf16 window 1MB + enc bf16 8.4MB + W 5.1MB.
"""

import numpy as np

B, S, D = 16, 2048, 1024
NCORES = 8
BL = B // NCORES
WSCALE = 32.0
CBF = 512         # s-prefix computed in bf16

_NC_CACHE = {}


def _build_program(bl, s, d, stage="all"):
    import concourse.bacc as bacc
    import concourse.bass as bass
    import concourse.mybir as mybir
    import concourse.tile as tile

    f32 = mybir.dt.float32
    bf16 = mybir.dt.bfloat16
    f8 = mybir.dt.float8e4
    i32 = mybir.dt.int32
    Tanh = mybir.ActivationFunctionType.Tanh
    Exp = mybir.ActivationFunctionType.Exp
    Identity = mybir.ActivationFunctionType.Identity
    Alu = mybir.AluOpType
    DR = mybir.MatmulPerfMode.DoubleRow

    kcn = d // 256       # fp8 DoubleRow K chunks (pairs on 128 partitions)
    dcn = d // 128       # bf16 K chunks
    ecn = d // 128       # e chunks
    sc2 = s // 128       # score chunks (s = f*128 + p layout)
    s8 = s - CBF         # fp8-region length
    DESCALE = 1.0 / WSCALE

    nc = bacc.Bacc()
    encT8_d = nc.declare_dram_parameter("encT8", [bl, 128, kcn, 2, s8], f8, isOutput=False)
    encTbf_d = nc.declare_dram_parameter("encTbf", [bl, 128, dcn, CBF], bf16, isOutput=False)
    encbf_d = nc.declare_dram_parameter("encbf", [bl, s, d], bf16, isOutput=False)
    wt8_d = nc.declare_dram_parameter("wt8", [128, kcn, 2, d], f8, isOutput=False)
    wtbf_d = nc.declare_dram_parameter("wtbf", [128, dcn, d], bf16, isOutput=False)
    whbf_d = nc.declare_dram_parameter("whbf", [128, dcn, d], bf16, isOutput=False)
    hidT_d = nc.declare_dram_parameter("hidT", [128, dcn, bl], bf16, isOutput=False)
    bcol_d = nc.declare_dram_parameter("bcol", [128, ecn], f32, isOutput=False)
    vcol_d = nc.declare_dram_parameter("vcol", [128, ecn], f32, isOutput=False)
    len_d = nc.declare_dram_parameter("len_i", [128, bl], i32, isOutput=False)
    if stage == "all":
        out_d = nc.declare_dram_parameter("ctx_out", [bl, d], f32, isOutput=True)
    else:
        out_d = nc.declare_dram_parameter("ctx_out", [bl, s], f32, isOutput=True)

    with tile.TileContext(nc) as tc:
        with (
            tc.tile_pool(name="consts", bufs=1) as consts,
            tc.tile_pool(name="etp", bufs=6) as etp,
            tc.tile_pool(name="enp", bufs=4) as enp,
            tc.tile_pool(name="p2p", bufs=6) as p2p,
            tc.tile_pool(name="sb1", bufs=1) as sb1,
            tc.tile_pool(name="psA", bufs=2, space="PSUM") as psA,
            tc.tile_pool(name="psS", bufs=2, space="PSUM") as psS,
            tc.tile_pool(name="psM", bufs=1, space="PSUM") as psM,
        ):
            # ---------------- startup DMAs (order = HBM priority) ----------
            # Sync queue feeds the pass-1 matmuls in consumption order
            # (fp8 s-half of batch 0 first -- it needs the fewest bytes);
            # Wh streams concurrently on the otherwise-idle Pool queue so
            # the bias chain is ready by the first tanh.
            hidT_sb = consts.tile([128, dcn, bl], bf16)
            nc.sync.dma_start(out=hidT_sb, in_=hidT_d[:, :, :])
            vcol_sb = consts.tile([128, ecn], f32)
            nc.sync.dma_start(out=vcol_sb, in_=vcol_d[:, :])
            bcol_sb = consts.tile([128, ecn], f32)
            nc.sync.dma_start(out=bcol_sb, in_=bcol_d[:, :])
            len_i_sb = consts.tile([128, bl], i32)
            nc.sync.dma_start(out=len_i_sb, in_=len_d[:, :])
            # Wh rides in pass-2 pool slots: read only by the hid matmul
            # group at startup, then the slots recycle into en2 tiles.
            wh_t = []
            for h in range(2):
                wh = p2p.tile([128, dcn // 2, d], bf16, tag="en2", name=f"wh{h}")
                nc.gpsimd.dma_start(
                    out=wh, in_=whbf_d[:, h * (dcn // 2):(h + 1) * (dcn // 2), :]
                )
                wh_t.append(wh)
            wt8_sb = consts.tile([128, kcn, 2, d], f8)
            nc.sync.dma_start(out=wt8_sb, in_=wt8_d[:, :, :, :])

            pre_et = {}

            def fetch_tiles(bb, sh1_first):
                def f_sh1():
                    e81 = etp.tile([128, kcn, 2, 1024], f8, tag="et", name=f"e81_{bb}")
                    nc.sync.dma_start(out=e81, in_=encT8_d[bb, :, :, :, 1024 - CBF:s8])
                    pre_et[(bb, 1)] = (None, e81)

                def f_sh0():
                    ebf = etp.tile([128, dcn, CBF], bf16, tag="et", name=f"ebf{bb}")
                    nc.sync.dma_start(out=ebf, in_=encTbf_d[bb, :, :, :])
                    e80 = etp.tile([128, kcn, 2, 1024 - CBF], f8, tag="et", name=f"e80_{bb}")
                    nc.sync.dma_start(out=e80, in_=encT8_d[bb, :, :, :, 0:1024 - CBF])
                    pre_et[(bb, 0)] = (ebf, e80)

                if sh1_first:
                    f_sh1()
                    f_sh0()
                else:
                    f_sh0()
                    f_sh1()

            fetch_tiles(0, sh1_first=True)
            wtbf_sb = consts.tile([128, dcn, d], bf16)
            nc.sync.dma_start(out=wtbf_sb, in_=wtbf_d[:, :, :])

            # ---------------- small consts ----------------
            len_f_sb = consts.tile([128, bl], f32)
            nc.vector.tensor_copy(len_f_sb, len_i_sb)
            iotaT_i = consts.tile([128, sc2], i32)
            nc.gpsimd.iota(
                iotaT_i, pattern=[[128, sc2]], base=0, channel_multiplier=1
            )
            iotaT_f = consts.tile([128, sc2], f32)
            nc.vector.tensor_copy(iotaT_f, iotaT_i)
            ones_sb = consts.tile([128, 1], f32)
            nc.vector.memset(ones_sb, 1.0)
            ones_bf = consts.tile([128, 1], bf16)
            nc.vector.memset(ones_bf, 1.0)
            ones_row = consts.tile([1, 128], f32)
            nc.vector.memset(ones_row, 1.0)
            # Upper bound M = sum|v| >= any score (|tanh|<=1), used instead
            # of the true max in softmax -- removes the serial max-reduce.
            vabs = consts.tile([128, 1], f32)
            nc.vector.reduce_sum(
                out=vabs, in_=vcol_sb, axis=mybir.AxisListType.X,
                apply_absolute_value=True,
            )
            psv = psS.tile([128, 16], f32, tag="s", name="psv")
            nc.tensor.matmul(
                psv[0:1, 0:1], vabs, ones_sb[:, 0:1], start=True, stop=True
            )
            mtot = consts.tile([1, 1], f32)
            nc.vector.tensor_copy(mtot, psv[0:1, 0:1])
            psb = psS.tile([128, 16], f32, tag="s", name="psb")
            nc.tensor.matmul(
                psb[:, 0:1], ones_row[:, :], mtot[:, :], start=True, stop=True
            )
            negM_bc = consts.tile([128, 1], f32)
            nc.scalar.mul(negM_bc, psb[:, 0:1], -1.0)
            validT = []
            for b_ in range(bl):
                vv = consts.tile([128, sc2], f32, name=f"validT{b_}")
                nc.vector.tensor_scalar(
                    vv, iotaT_f, len_f_sb[:, b_:b_ + 1], None, op0=Alu.is_lt
                )
                validT.append(vv)

            # ---------------- hid_proj + b -> bias_all[:, ec, b] -----------
            # Transposed bf16 hid projection: one PSUM accumulation group
            # of dcn*ecn matmuls, each writing its own [128, bl] region of
            # a single bank -- no DRAM bounce needed to get the
            # [e-partition] layout the tanh bias wants.
            ps_hbT = psS.tile([128, 16], f32, tag="s", name="ps_hbT")
            for kc in range(dcn):
                for ec in range(ecn):
                    nc.tensor.matmul(
                        ps_hbT[:, ec * bl:(ec + 1) * bl],
                        wh_t[kc // 4][:, kc % 4, ec * 128:(ec + 1) * 128],
                        hidT_sb[:, kc, :],
                        start=(kc == 0 and ec == 0),
                        stop=(kc == dcn - 1 and ec == ecn - 1),
                        skip_group_check=True,
                    )
            bias_all = consts.tile([128, ecn, bl], f32)
            for ec in range(ecn):
                nc.scalar.activation(
                    bias_all[:, ec, :],
                    ps_hbT[:, ec * bl:(ec + 1) * bl],
                    Identity,
                    bias=bcol_sb[:, ec:ec + 1],
                )

            scoresT = [
                consts.tile([128, sc2], f32, name=f"scoresT{b_}")
                for b_ in range(bl)
            ]
            attnT = [
                consts.tile([128, sc2], bf16, name=f"attnT{b_}")
                for b_ in range(bl)
            ]

            # ---------------- pass 1: scores ----------------
            def flush_pending(pending):
                # Deferred partition-reduce of the v-dot acc tile: one
                # 8-matmul PSUM group (each chunk c of acc column-sums into
                # its own column of a single bank) + one DVE copy out.
                acc_p, bb_p, sh_p = pending
                sps = psS.tile([128, 16], f32, tag="s")
                for c_ in range(8):
                    nc.tensor.matmul(
                        sps[:, c_:c_ + 1],
                        acc_p[:, c_ * 128:(c_ + 1) * 128],
                        ones_bf[:, 0:1],
                        start=(c_ == 0),
                        stop=(c_ == 7),
                        skip_group_check=True,
                    )
                nc.vector.tensor_copy(
                    scoresT[bb_p][:, sh_p * 8:(sh_p + 1) * 8], sps[:, 0:8]
                )

            def softmax(bb):
                # exp(score - M), mask + per-partition row-sum fused in one
                # DVE pass, then a 128->1 matmul for the total.
                attn_raw = sb1.tile([128, sc2], f32, tag="araw")
                nc.scalar.activation(
                    attn_raw, scoresT[bb], Exp, bias=negM_bc[:, 0:1]
                )
                attn_exp = sb1.tile([128, sc2], f32, tag="aexp")
                psums = sb1.tile([128, 1], f32, tag="psums")
                nc.vector.scalar_tensor_tensor(
                    attn_exp,
                    attn_raw,
                    1.0,
                    validT[bb],
                    op0=Alu.mult,
                    op1=Alu.mult,
                    accum_out=psums,
                )
                nc.vector.tensor_copy(attnT[bb], attn_exp)
                psm = psS.tile([128, 16], f32, tag="s")
                nc.tensor.matmul(
                    psm[0:1, 0:1], psums, ones_sb[:, 0:1], start=True, stop=True
                )
                rinv = sb1.tile([1, 1], f32, tag=f"rinv{bb}", name=f"rinv{bb}")
                nc.vector.reciprocal(rinv, psm[0:1, 0:1])
                if stage == "sm":
                    nc.gpsimd.dma_start(
                        out=out_d[bb, :].rearrange("(f p) -> p f", p=128),
                        in_=attn_exp,
                    )
                return rinv

            def p1_tile(bb, sh, pending):
                # One [128(e), 1024(s)] energy^T tile per ec: for sh=0 the
                # first bank is the bf16 window (s < CBF), the second bank
                # fp8; sh=1 is all fp8.  Both weight variants carry the x32
                # scale, so one tanh descale covers the whole tile.
                ebf, e8 = pre_et.pop((bb, sh))
                acc = enp.tile([128, 1024], bf16, tag="acc", bufs=3)
                for ec in range(ecn):
                    ps = psA.tile([128, 1024], f32, tag="proj")
                    if sh == 0:
                        for kc in range(dcn):
                            nc.tensor.matmul(
                                ps[:, 0:CBF],
                                wtbf_sb[:, kc, ec * 128:(ec + 1) * 128],
                                ebf[:, kc, :],
                                start=(kc == 0),
                                stop=(kc == dcn - 1),
                                skip_group_check=True,
                            )
                        f8_banks = [(CBF, 0)]
                    else:
                        f8_banks = [(0, 0), (512, 512)]
                    # fp8 DoubleRow: moving operands cap at 512 src elements
                    # = 256 out columns, so each 512-wide PSUM bank is one
                    # accumulation group of 4kc x 2 quarter matmuls.
                    for p0, s0 in f8_banks:
                        for kc in range(kcn):
                            for q in range(2):
                                nc.tensor.matmul(
                                    ps[:, p0 + q * 256:p0 + (q + 1) * 256],
                                    wt8_sb[:, kc, :, ec * 128:(ec + 1) * 128],
                                    e8[:, kc, :, s0 + q * 256:s0 + (q + 1) * 256],
                                    start=(kc == 0 and q == 0),
                                    stop=(kc == kcn - 1 and q == 1),
                                    perf_mode=DR,
                                    skip_group_check=True,
                                )
                    if ec == 2 and pending is not None:
                        flush_pending(pending)
                        pending = None
                    en = enp.tile([128, 1024], bf16, tag="en")
                    nc.scalar.activation(
                        en, ps, Tanh, bias=bias_all[:, ec, bb:bb + 1],
                        scale=DESCALE,
                    )
                    # v-dot on DVE: acc[p, s] += v[ec*128+p] * en[p, s];
                    # all-bf16 SBUF operands keep the DVE in its fast mode.
                    if ec == 0:
                        nc.vector.tensor_scalar_mul(acc, en, vcol_sb[:, 0:1])
                    else:
                        nc.vector.scalar_tensor_tensor(
                            acc,
                            en,
                            vcol_sb[:, ec:ec + 1],
                            acc,
                            op0=Alu.mult,
                            op1=Alu.add,
                        )
                if pending is not None:
                    flush_pending(pending)
                return (acc, bb, sh)

            en2_tiles = {}

            def fetch_en2(bb):
                for gi in range(4):
                    en2 = p2p.tile([128, 4, d], bf16, tag="en2")
                    nc.sync.dma_start(
                        out=en2,
                        in_=encbf_d[bb, gi * 512:(gi + 1) * 512, :].rearrange(
                            "(j p) e -> p j e", p=128
                        ),
                    )
                    en2_tiles[(bb, gi)] = en2

            def p2_group(bb, gi, cps):
                en2 = en2_tiles.pop((bb, gi))
                for j in range(4):
                    sci = gi * 4 + j
                    for h in range(2):
                        nc.tensor.matmul(
                            cps[:, h * 512:(h + 1) * 512],
                            attnT[bb][:, sci:sci + 1],
                            en2[:, j, h * 512:(h + 1) * 512],
                            start=(sci == 0),
                            stop=(sci == sc2 - 1),
                            skip_group_check=True,
                        )

            def p2_finish(bb, cps, rinv):
                ctx = sb1.tile([1, d], f32, tag=f"ctx{bb}", name=f"ctx{bb}")
                nc.vector.tensor_scalar_mul(ctx, cps, rinv[0:1, 0:1])
                nc.gpsimd.dma_start(out=out_d[bb:bb + 1, :], in_=ctx)

            # Emission order is PE execution order: pass-1 tiles ordered by
            # DMA-readiness (fp8-only halves need the fewest early bytes),
            # flushes deferred into the next tile's matmul stream, softmax
            # slotted where its tiny PE ops find their inputs long-ready,
            # pass-2 of b0 split around the last flush.  All pass-2 and
            # next-batch DMAs are issued well ahead of use.
            pending = p1_tile(0, 1, None)
            fetch_tiles(1, sh1_first=False)
            pending = p1_tile(0, 0, pending)
            fetch_en2(0)
            pending = p1_tile(1, 1, pending)
            rinv0 = softmax(0)
            pending = p1_tile(1, 0, pending)
            fetch_en2(1)
            if stage in ("p1", "sm"):
                flush_pending(pending)
                rinv1 = softmax(1)
                if stage == "p1":
                    for bb in range(bl):
                        nc.gpsimd.dma_start(
                            out=out_d[bb, :].rearrange("(f p) -> p f", p=128),
                            in_=scoresT[bb],
                        )
            else:
                cps0 = psM.tile([1, d], f32, tag="m", name="cps0")
                p2_group(0, 0, cps0)
                p2_group(0, 1, cps0)
                flush_pending(pending)
                p2_group(0, 2, cps0)
                p2_group(0, 3, cps0)
                rinv1 = softmax(1)
                p2_finish(0, cps0, rinv0)
                cps1 = psM.tile([1, d], f32, tag="m", name="cps1")
                for gi in range(4):
                    p2_group(1, gi, cps1)
                p2_finish(1, cps1, rinv1)

    nc.compile()
    return nc


def _get_nc(bl=BL, s=S, d=D, stage="all"):
    key = (bl, s, d, stage)
    if key not in _NC_CACHE:
        _NC_CACHE[key] = _build_program(bl, s, d, stage)
    return _NC_CACHE[key]


def _to_e4m3(x, scale=1.0):
    import ml_dtypes

    return np.clip(
        np.asarray(x, dtype=np.float32) * scale, -240.0, 240.0
    ).astype(ml_dtypes.float8_e4m3)


def _pairchunk(xT):
    """[d, n] -> [p, kc, i, n] with d = kc*256 + p*2 + i."""
    d, n = xT.shape
    return np.ascontiguousarray(
        xT.reshape(d // 256, 128, 2, n).transpose(1, 0, 2, 3)
    )


def _chunk(xT):
    """[d, n] -> [p, kc, n] with d = kc*128 + p."""
    d, n = xT.shape
    return np.ascontiguousarray(xT.reshape(d // 128, 128, n).transpose(1, 0, 2))


def _make_in_maps(encoder_outputs, hidden, lengths, W, b, v):
    import ml_dtypes

    BF = ml_dtypes.bfloat16
    enc = np.asarray(encoder_outputs, dtype=np.float32)
    hid = np.asarray(hidden, dtype=np.float32)
    len_ = np.asarray(lengths, dtype=np.int32)
    Wn = np.asarray(W, dtype=np.float32)
    bn = np.asarray(b, dtype=np.float32)
    vn = np.asarray(v, dtype=np.float32)

    ecn = D // 128
    Wh, We = Wn[:, :D], Wn[:, D:]                        # [e, d] each
    wt8 = _pairchunk(_to_e4m3(We, WSCALE).T)
    wtbf = _chunk((We.T * WSCALE).astype(BF))
    whbf = _chunk(Wh.T.astype(BF))
    bcol = np.ascontiguousarray(bn.reshape(ecn, 128).T)  # [128, ecn]
    vcol = np.ascontiguousarray(vn.reshape(ecn, 128).T)
    enc8 = _to_e4m3(enc)                                 # [B, s, d]
    encbf = enc.astype(BF)
    in_maps = []
    for i in range(NCORES):
        sl = slice(BL * i, BL * (i + 1))
        in_maps.append(
            dict(
                encT8=np.stack(
                    [_pairchunk(enc8[j, CBF:].T) for j in range(sl.start, sl.stop)]
                ),
                encTbf=np.stack(
                    [_chunk(encbf[j, :CBF].T) for j in range(sl.start, sl.stop)]
                ),
                encbf=np.ascontiguousarray(encbf[sl]),
                wt8=wt8,
                wtbf=wtbf,
                whbf=whbf,
                hidT=_chunk(np.ascontiguousarray(hid[sl].T.astype(BF))),
                bcol=bcol,
                vcol=vcol,
                len_i=np.ascontiguousarray(
                    np.broadcast_to(len_[sl].reshape(1, BL), (128, BL)).copy()
                ),
            )
        )
    return in_maps


def run(inputs, trace=False, stage="all"):
    """Run on 8 NeuronCores; returns (output [B,1,D], BassKernelResults)."""
    from concourse.bass_utils import run_bass_kernel_spmd

    nc = _get_nc(stage=stage)
    in_maps = _make_in_maps(**inputs)
    r = run_bass_kernel_spmd(
        nc, in_maps, core_ids=list(range(NCORES)), trace=trace
    )
    out = np.concatenate(
        [np.asarray(r.results[i]["ctx_out"]) for i in range(NCORES)], axis=0
    )
    if stage != "all":
        return out, r
    return out[:, None, :].astype(np.float32), r


def kernel(encoder_outputs, hidden, lengths, W, b, v):
    out, _ = run(
        dict(
            encoder_outputs=encoder_outputs,
            hidden=hidden,
            lengths=lengths,
            W=W,
            b=b,
            v=v,
        )
    )
    return out


# revision 21
# speedup vs baseline: 1.0689x; 1.0689x over previous
"""Trainium2 Bass kernel for nn_Attn (additive/Bahdanau-style attention).

Math (per batch b):
    Wh, We   = W[:, :D], W[:, D:]                       # [D,D] each
    energy   = tanh(enc @ We.T + hidden @ Wh.T + b)     # [S, D]
    scores   = energy @ v, masked to length, softmax    # [S]
    context  = scores @ enc                             # [D]

Sharding: data-parallel over batch B=16 across 8 cores (2 batches/core);
W, b, v replicated.

Numerics / layout (validated offline against the reference inputs:
worst-batch rel_err ~7.8e-3 vs the 2e-2 gate):
  - Pass-1 enc_proj runs mostly in fp8 e4m3 with MatmulPerfMode.DoubleRow
    (2 fp8 K-values per PE cell -> 2x bf16 throughput).  Quantization
    noise on the scores (~0.03 absolute) fails short-`lengths` batches
    whose softmax support is too small to average it out, so s < 512 is
    computed in bf16 instead: short batches become fully bf16-accurate
    and long batches keep enough support for the fp8 noise to wash out.
  - Both We variants are pre-scaled by 32 host-side (fp8: clears the
    e4m3 subnormal range; bf16: scale-free) so one 1/32 descale in the
    tanh activation covers every pass-1 tile.  fp8 K = 1024 maps to 4
    chunks of 256 with d = kc*256 + p*2 + i (pair axis = dim 1).
  - energy^T tiles are [e=128, s=1024] (two single-bank PSUM matmul
    groups per tile) so each tanh is one big ACT op with the per-batch
    bias (hid_proj + b) as a per-partition scalar.
  - hid_proj is computed transposed ([e-part, batch], bf16) as one
    64-matmul PSUM accumulation group -> bias needs no DRAM bounce.
  - The v-dot runs on the DVE as an all-bf16 scalar_tensor_tensor chain
    (2-byte SBUF operands unlock the DVE fast modes); the 128->1
    partition reduce is one 8-matmul PSUM group per [128, 1024] acc tile.
  - Scores live in [128, S/128] layout; masked softmax uses the static
    bound M = sum|v| >= max(score) instead of a max-reduce; the exp
    table is accurate even at exp(-25) (verified on hw); normalization
    by 1/sum is applied to the final context vector.
  - Pass 2 (context) re-loads enc in natural [s, d] layout as bf16, all
    tiles DMA'd ahead, and runs accumulating matmuls (attn column
    stationary, enc moving).
  - HBM/core: enc fp8 3.1MB + bf16 window 1MB + enc bf16 8.4MB + W 5.1MB.
"""

import numpy as np

B, S, D = 16, 2048, 1024
NCORES = 8
BL = B // NCORES
WSCALE = 32.0
CBF = 512         # s-prefix computed in bf16

_NC_CACHE = {}


def _build_program(bl, s, d, stage="all"):
    import concourse.bacc as bacc
    import concourse.bass as bass
    import concourse.mybir as mybir
    import concourse.tile as tile

    f32 = mybir.dt.float32
    bf16 = mybir.dt.bfloat16
    f8 = mybir.dt.float8e4
    i32 = mybir.dt.int32
    Tanh = mybir.ActivationFunctionType.Tanh
    Exp = mybir.ActivationFunctionType.Exp
    Identity = mybir.ActivationFunctionType.Identity
    Alu = mybir.AluOpType
    DR = mybir.MatmulPerfMode.DoubleRow

    kcn = d // 256       # fp8 DoubleRow K chunks (pairs on 128 partitions)
    dcn = d // 128       # bf16 K chunks
    ecn = d // 128       # e chunks
    sc2 = s // 128       # score chunks (s = f*128 + p layout)
    s8 = s - CBF         # fp8-region length
    DESCALE = 1.0 / WSCALE

    nc = bacc.Bacc()
    encT8_d = nc.declare_dram_parameter("encT8", [bl, 128, kcn, 2, s8], f8, isOutput=False)
    encTbf_d = nc.declare_dram_parameter("encTbf", [bl, 128, dcn, CBF], bf16, isOutput=False)
    encbf_d = nc.declare_dram_parameter("encbf", [bl, s, d], bf16, isOutput=False)
    wt8_d = nc.declare_dram_parameter("wt8", [128, kcn, 2, d], f8, isOutput=False)
    wtbf_d = nc.declare_dram_parameter("wtbf", [128, dcn, d], bf16, isOutput=False)
    whbf_d = nc.declare_dram_parameter("whbf", [128, dcn, d], bf16, isOutput=False)
    hidT_d = nc.declare_dram_parameter("hidT", [128, dcn, bl], bf16, isOutput=False)
    bcol_d = nc.declare_dram_parameter("bcol", [128, ecn], f32, isOutput=False)
    vcol_d = nc.declare_dram_parameter("vcol", [128, ecn], f32, isOutput=False)
    len_d = nc.declare_dram_parameter("len_i", [128, bl], i32, isOutput=False)
    if stage == "all":
        out_d = nc.declare_dram_parameter("ctx_out", [bl, d], f32, isOutput=True)
    else:
        out_d = nc.declare_dram_parameter("ctx_out", [bl, s], f32, isOutput=True)

    with tile.TileContext(nc) as tc:
        with (
            tc.tile_pool(name="consts", bufs=1) as consts,
            tc.tile_pool(name="etp", bufs=6) as etp,
            tc.tile_pool(name="enp", bufs=4) as enp,
            tc.tile_pool(name="p2p", bufs=6) as p2p,
            tc.tile_pool(name="sb1", bufs=1) as sb1,
            tc.tile_pool(name="psA", bufs=2, space="PSUM") as psA,
            tc.tile_pool(name="psS", bufs=2, space="PSUM") as psS,
            tc.tile_pool(name="psM", bufs=1, space="PSUM") as psM,
        ):
            # ---------------- startup DMAs (order = HBM priority) ----------
            # Sync queue feeds the pass-1 matmuls in consumption order
            # (fp8 s-half of batch 0 first -- it needs the fewest bytes);
            # Wh streams concurrently on the otherwise-idle Pool queue so
            # the bias chain is ready by the first tanh.
            hidT_sb = consts.tile([128, dcn, bl], bf16)
            nc.sync.dma_start(out=hidT_sb, in_=hidT_d[:, :, :])
            # Wh rides in pass-2 pool slots: read only by the hid matmul
            # group at startup, then the slots recycle into en2 tiles.
            wh_t = []
            for h in range(2):
                wh = p2p.tile([128, dcn // 2, d], bf16, tag="en2", name=f"wh{h}")
                nc.gpsimd.dma_start(
                    out=wh, in_=whbf_d[:, h * (dcn // 2):(h + 1) * (dcn // 2), :]
                )
                wh_t.append(wh)
            wt8_sb = consts.tile([128, kcn, 2, d], f8)
            nc.sync.dma_start(out=wt8_sb, in_=wt8_d[:, :, :, :])

            pre_et = {}

            def fetch_tiles(bb, sh1_first):
                def f_sh1():
                    e81 = etp.tile([128, kcn, 2, 1024], f8, tag="et", name=f"e81_{bb}")
                    nc.sync.dma_start(out=e81, in_=encT8_d[bb, :, :, :, 1024 - CBF:s8])
                    pre_et[(bb, 1)] = (None, e81)

                def f_sh0():
                    ebf = etp.tile([128, dcn, CBF], bf16, tag="et", name=f"ebf{bb}")
                    nc.sync.dma_start(out=ebf, in_=encTbf_d[bb, :, :, :])
                    e80 = etp.tile([128, kcn, 2, 1024 - CBF], f8, tag="et", name=f"e80_{bb}")
                    nc.sync.dma_start(out=e80, in_=encT8_d[bb, :, :, :, 0:1024 - CBF])
                    pre_et[(bb, 0)] = (ebf, e80)

                if sh1_first:
                    f_sh1()
                    f_sh0()
                else:
                    f_sh0()
                    f_sh1()

            fetch_tiles(0, sh1_first=True)
            wtbf_sb = consts.tile([128, dcn, d], bf16)
            nc.sync.dma_start(out=wtbf_sb, in_=wtbf_d[:, :, :])
            vcol_sb = consts.tile([128, ecn], f32)
            nc.sync.dma_start(out=vcol_sb, in_=vcol_d[:, :])
            bcol_sb = consts.tile([128, ecn], f32)
            nc.sync.dma_start(out=bcol_sb, in_=bcol_d[:, :])
            len_i_sb = consts.tile([128, bl], i32)
            nc.sync.dma_start(out=len_i_sb, in_=len_d[:, :])

            # ---------------- small consts ----------------
            len_f_sb = consts.tile([128, bl], f32)
            nc.vector.tensor_copy(len_f_sb, len_i_sb)
            iotaT_i = consts.tile([128, sc2], i32)
            nc.gpsimd.iota(
                iotaT_i, pattern=[[128, sc2]], base=0, channel_multiplier=1
            )
            iotaT_f = consts.tile([128, sc2], f32)
            nc.vector.tensor_copy(iotaT_f, iotaT_i)
            ones_sb = consts.tile([128, 1], f32)
            nc.vector.memset(ones_sb, 1.0)
            ones_bf = consts.tile([128, 1], bf16)
            nc.vector.memset(ones_bf, 1.0)
            ones_row = consts.tile([1, 128], f32)
            nc.vector.memset(ones_row, 1.0)
            validT = []
            for b_ in range(bl):
                vv = consts.tile([128, sc2], f32, name=f"validT{b_}")
                nc.vector.tensor_scalar(
                    vv, iotaT_f, len_f_sb[:, b_:b_ + 1], None, op0=Alu.is_lt
                )
                validT.append(vv)

            # ---------------- hid_proj + b -> bias_all[:, ec, b] -----------
            # Transposed bf16 hid projection: one PSUM accumulation group
            # of dcn*ecn matmuls, each writing its own [128, bl] region of
            # a single bank -- no DRAM bounce needed to get the
            # [e-partition] layout the tanh bias wants.  The matmul group
            # is emitted MID-way through the first pass-1 tile (via mid_cb)
            # so its Wh-gated, ldweights-paced crawl overlaps real work;
            # the bias ACTs stay ahead of the first tanh on the ACT queue.
            ps_hbT = psS.tile([128, 16], f32, tag="s", name="ps_hbT")
            bias_all = consts.tile([128, ecn, bl], f32)

            def emit_hid_mms():
                for kc in range(dcn):
                    for ec in range(ecn):
                        nc.tensor.matmul(
                            ps_hbT[:, ec * bl:(ec + 1) * bl],
                            wh_t[kc // 4][:, kc % 4, ec * 128:(ec + 1) * 128],
                            hidT_sb[:, kc, :],
                            start=(kc == 0 and ec == 0),
                            stop=(kc == dcn - 1 and ec == ecn - 1),
                            skip_group_check=True,
                        )
                for ec in range(ecn):
                    nc.scalar.activation(
                        bias_all[:, ec, :],
                        ps_hbT[:, ec * bl:(ec + 1) * bl],
                        Identity,
                        bias=bcol_sb[:, ec:ec + 1],
                    )

            def emit_negM():
                # Upper bound M = sum|v| >= any score (|tanh|<=1), used
                # instead of the true max in softmax -- no max-reduce.
                vabs = consts.tile([128, 1], f32)
                nc.vector.reduce_sum(
                    out=vabs, in_=vcol_sb, axis=mybir.AxisListType.X,
                    apply_absolute_value=True,
                )
                psv = psS.tile([128, 16], f32, tag="s", name="psv")
                nc.tensor.matmul(
                    psv[0:1, 0:1], vabs, ones_sb[:, 0:1], start=True, stop=True
                )
                mtot = consts.tile([1, 1], f32)
                nc.vector.tensor_copy(mtot, psv[0:1, 0:1])
                psb = psS.tile([128, 16], f32, tag="s", name="psb")
                nc.tensor.matmul(
                    psb[:, 0:1], ones_row[:, :], mtot[:, :], start=True, stop=True
                )
                negM_bc = consts.tile([128, 1], f32)
                nc.scalar.mul(negM_bc, psb[:, 0:1], -1.0)
                return negM_bc

            scoresT = [
                consts.tile([128, sc2], f32, name=f"scoresT{b_}")
                for b_ in range(bl)
            ]
            attnT = [
                consts.tile([128, sc2], bf16, name=f"attnT{b_}")
                for b_ in range(bl)
            ]

            # ---------------- pass 1: scores ----------------
            def flush_pending(pending):
                # Deferred partition-reduce of the v-dot acc tile: one
                # 8-matmul PSUM group (each chunk c of acc column-sums into
                # its own column of a single bank) + one DVE copy out.
                acc_p, bb_p, sh_p = pending
                sps = psS.tile([128, 16], f32, tag="s")
                for c_ in range(8):
                    nc.tensor.matmul(
                        sps[:, c_:c_ + 1],
                        acc_p[:, c_ * 128:(c_ + 1) * 128],
                        ones_bf[:, 0:1],
                        start=(c_ == 0),
                        stop=(c_ == 7),
                        skip_group_check=True,
                    )
                nc.vector.tensor_copy(
                    scoresT[bb_p][:, sh_p * 8:(sh_p + 1) * 8], sps[:, 0:8]
                )

            def softmax(bb, negM_bc):
                # exp(score - M), mask + per-partition row-sum fused in one
                # DVE pass, then a 128->1 matmul for the total.
                attn_raw = sb1.tile([128, sc2], f32, tag="araw")
                nc.scalar.activation(
                    attn_raw, scoresT[bb], Exp, bias=negM_bc[:, 0:1]
                )
                attn_exp = sb1.tile([128, sc2], f32, tag="aexp")
                psums = sb1.tile([128, 1], f32, tag="psums")
                nc.vector.scalar_tensor_tensor(
                    attn_exp,
                    attn_raw,
                    1.0,
                    validT[bb],
                    op0=Alu.mult,
                    op1=Alu.mult,
                    accum_out=psums,
                )
                nc.vector.tensor_copy(attnT[bb], attn_exp)
                psm = psS.tile([128, 16], f32, tag="s")
                nc.tensor.matmul(
                    psm[0:1, 0:1], psums, ones_sb[:, 0:1], start=True, stop=True
                )
                rinv = sb1.tile([1, 1], f32, tag=f"rinv{bb}", name=f"rinv{bb}")
                nc.vector.reciprocal(rinv, psm[0:1, 0:1])
                if stage == "sm":
                    nc.gpsimd.dma_start(
                        out=out_d[bb, :].rearrange("(f p) -> p f", p=128),
                        in_=attn_exp,
                    )
                return rinv

            def p1_tile(bb, sh, pending, mid_cb=None):
                # One [128(e), 1024(s)] energy^T tile per ec: for sh=0 the
                # first bank is the bf16 window (s < CBF), the second bank
                # fp8; sh=1 is all fp8.  Both weight variants carry the x32
                # scale, so one tanh descale covers the whole tile.
                ebf, e8 = pre_et.pop((bb, sh))
                acc = enp.tile([128, 1024], bf16, tag="acc", bufs=3)
                for ec in range(ecn):
                    ps = psA.tile([128, 1024], f32, tag="proj")
                    if sh == 0:
                        for kc in range(dcn):
                            nc.tensor.matmul(
                                ps[:, 0:CBF],
                                wtbf_sb[:, kc, ec * 128:(ec + 1) * 128],
                                ebf[:, kc, :],
                                start=(kc == 0),
                                stop=(kc == dcn - 1),
                                skip_group_check=True,
                            )
                        f8_banks = [(CBF, 0)]
                    else:
                        f8_banks = [(0, 0), (512, 512)]
                    # fp8 DoubleRow: moving operands cap at 512 src elements
                    # = 256 out columns, so each 512-wide PSUM bank is one
                    # accumulation group of 4kc x 2 quarter matmuls.
                    for p0, s0 in f8_banks:
                        for kc in range(kcn):
                            for q in range(2):
                                nc.tensor.matmul(
                                    ps[:, p0 + q * 256:p0 + (q + 1) * 256],
                                    wt8_sb[:, kc, :, ec * 128:(ec + 1) * 128],
                                    e8[:, kc, :, s0 + q * 256:s0 + (q + 1) * 256],
                                    start=(kc == 0 and q == 0),
                                    stop=(kc == kcn - 1 and q == 1),
                                    perf_mode=DR,
                                    skip_group_check=True,
                                )
                    if ec == 2 and pending is not None:
                        flush_pending(pending)
                        pending = None
                    if mid_cb is not None:
                        # emitted after ec0's matmul groups, before the
                        # first tanh that consumes what it produces
                        mid_cb()
                        mid_cb = None
                    en = enp.tile([128, 1024], bf16, tag="en")
                    nc.scalar.activation(
                        en, ps, Tanh, bias=bias_all[:, ec, bb:bb + 1],
                        scale=DESCALE,
                    )
                    # v-dot on DVE: acc[p, s] += v[ec*128+p] * en[p, s];
                    # all-bf16 SBUF operands keep the DVE in its fast mode.
                    if ec == 0:
                        nc.vector.tensor_scalar_mul(acc, en, vcol_sb[:, 0:1])
                    else:
                        nc.vector.scalar_tensor_tensor(
                            acc,
                            en,
                            vcol_sb[:, ec:ec + 1],
                            acc,
                            op0=Alu.mult,
                            op1=Alu.add,
                        )
                if pending is not None:
                    flush_pending(pending)
                return (acc, bb, sh)

            en2_tiles = {}

            def fetch_en2(bb):
                for gi in range(4):
                    en2 = p2p.tile([128, 4, d], bf16, tag="en2")
                    nc.sync.dma_start(
                        out=en2,
                        in_=encbf_d[bb, gi * 512:(gi + 1) * 512, :].rearrange(
                            "(j p) e -> p j e", p=128
                        ),
                    )
                    en2_tiles[(bb, gi)] = en2

            def p2_group(bb, gi, cps):
                en2 = en2_tiles.pop((bb, gi))
                for j in range(4):
                    sci = gi * 4 + j
                    for h in range(2):
                        nc.tensor.matmul(
                            cps[:, h * 512:(h + 1) * 512],
                            attnT[bb][:, sci:sci + 1],
                            en2[:, j, h * 512:(h + 1) * 512],
                            start=(sci == 0),
                            stop=(sci == sc2 - 1),
                            skip_group_check=True,
                        )

            def p2_finish(bb, cps, rinv):
                ctx = sb1.tile([1, d], f32, tag=f"ctx{bb}", name=f"ctx{bb}")
                nc.vector.tensor_scalar_mul(ctx, cps, rinv[0:1, 0:1])
                nc.gpsimd.dma_start(out=out_d[bb:bb + 1, :], in_=ctx)

            # Emission order is PE execution order: pass-1 tiles ordered by
            # DMA-readiness (fp8-only halves need the fewest early bytes),
            # flushes deferred into the next tile's matmul stream, softmax
            # slotted where its tiny PE ops find their inputs long-ready,
            # pass-2 of b0 split around the last flush.  All pass-2 and
            # next-batch DMAs are issued well ahead of use.
            pending = p1_tile(0, 1, None, mid_cb=emit_hid_mms)
            fetch_tiles(1, sh1_first=False)
            pending = p1_tile(0, 0, pending)
            fetch_en2(0)
            pending = p1_tile(1, 1, pending)
            negM_bc = emit_negM()
            rinv0 = softmax(0, negM_bc)
            pending = p1_tile(1, 0, pending)
            fetch_en2(1)
            if stage in ("p1", "sm"):
                flush_pending(pending)
                rinv1 = softmax(1, negM_bc)
                if stage == "p1":
                    for bb in range(bl):
                        nc.gpsimd.dma_start(
                            out=out_d[bb, :].rearrange("(f p) -> p f", p=128),
                            in_=scoresT[bb],
                        )
            else:
                cps0 = psM.tile([1, d], f32, tag="m", name="cps0")
                p2_group(0, 0, cps0)
                p2_group(0, 1, cps0)
                flush_pending(pending)
                p2_group(0, 2, cps0)
                p2_group(0, 3, cps0)
                rinv1 = softmax(1, negM_bc)
                p2_finish(0, cps0, rinv0)
                cps1 = psM.tile([1, d], f32, tag="m", name="cps1")
                for gi in range(4):
                    p2_group(1, gi, cps1)
                p2_finish(1, cps1, rinv1)

    nc.compile()
    return nc


def _get_nc(bl=BL, s=S, d=D, stage="all"):
    key = (bl, s, d, stage)
    if key not in _NC_CACHE:
        _NC_CACHE[key] = _build_program(bl, s, d, stage)
    return _NC_CACHE[key]


def _to_e4m3(x, scale=1.0):
    import ml_dtypes

    return np.clip(
        np.asarray(x, dtype=np.float32) * scale, -240.0, 240.0
    ).astype(ml_dtypes.float8_e4m3)


def _pairchunk(xT):
    """[d, n] -> [p, kc, i, n] with d = kc*256 + p*2 + i."""
    d, n = xT.shape
    return np.ascontiguousarray(
        xT.reshape(d // 256, 128, 2, n).transpose(1, 0, 2, 3)
    )


def _chunk(xT):
    """[d, n] -> [p, kc, n] with d = kc*128 + p."""
    d, n = xT.shape
    return np.ascontiguousarray(xT.reshape(d // 128, 128, n).transpose(1, 0, 2))


def _make_in_maps(encoder_outputs, hidden, lengths, W, b, v):
    import ml_dtypes

    BF = ml_dtypes.bfloat16
    enc = np.asarray(encoder_outputs, dtype=np.float32)
    hid = np.asarray(hidden, dtype=np.float32)
    len_ = np.asarray(lengths, dtype=np.int32)
    Wn = np.asarray(W, dtype=np.float32)
    bn = np.asarray(b, dtype=np.float32)
    vn = np.asarray(v, dtype=np.float32)

    ecn = D // 128
    Wh, We = Wn[:, :D], Wn[:, D:]                        # [e, d] each
    wt8 = _pairchunk(_to_e4m3(We, WSCALE).T)
    wtbf = _chunk((We.T * WSCALE).astype(BF))
    whbf = _chunk(Wh.T.astype(BF))
    bcol = np.ascontiguousarray(bn.reshape(ecn, 128).T)  # [128, ecn]
    vcol = np.ascontiguousarray(vn.reshape(ecn, 128).T)
    enc8 = _to_e4m3(enc)                                 # [B, s, d]
    encbf = enc.astype(BF)
    in_maps = []
    for i in range(NCORES):
        sl = slice(BL * i, BL * (i + 1))
        in_maps.append(
            dict(
                encT8=np.stack(
                    [_pairchunk(enc8[j, CBF:].T) for j in range(sl.start, sl.stop)]
                ),
                encTbf=np.stack(
                    [_chunk(encbf[j, :CBF].T) for j in range(sl.start, sl.stop)]
                ),
                encbf=np.ascontiguousarray(encbf[sl]),
                wt8=wt8,
                wtbf=wtbf,
                whbf=whbf,
                hidT=_chunk(np.ascontiguousarray(hid[sl].T.astype(BF))),
                bcol=bcol,
                vcol=vcol,
                len_i=np.ascontiguousarray(
                    np.broadcast_to(len_[sl].reshape(1, BL), (128, BL)).copy()
                ),
            )
        )
    return in_maps


def run(inputs, trace=False, stage="all"):
    """Run on 8 NeuronCores; returns (output [B,1,D], BassKernelResults)."""
    from concourse.bass_utils import run_bass_kernel_spmd

    nc = _get_nc(stage=stage)
    in_maps = _make_in_maps(**inputs)
    r = run_bass_kernel_spmd(
        nc, in_maps, core_ids=list(range(NCORES)), trace=trace
    )
    out = np.concatenate(
        [np.asarray(r.results[i]["ctx_out"]) for i in range(NCORES)], axis=0
    )
    if stage != "all":
        return out, r
    return out[:, None, :].astype(np.float32), r


def kernel(encoder_outputs, hidden, lengths, W, b, v):
    out, _ = run(
        dict(
            encoder_outputs=encoder_outputs,
            hidden=hidden,
            lengths=lengths,
            W=W,
            b=b,
            v=v,
        )
    )
    return out


# revision 25
# speedup vs baseline: 1.1548x; 1.0804x over previous
"""Trainium2 Bass kernel for nn_Attn (additive/Bahdanau-style attention).

Math (per batch b):
    Wh, We   = W[:, :D], W[:, D:]                       # [D,D] each
    energy   = tanh(enc @ We.T + hidden @ Wh.T + b)     # [S, D]
    scores   = energy @ v, masked to length, softmax    # [S]
    context  = scores @ enc                             # [D]

Sharding: data-parallel over batch B=16 across 8 cores (2 batches/core);
W, b, v replicated.

Numerics / layout (validated offline against the reference inputs:
worst-batch rel_err ~7.8e-3 vs the 2e-2 gate):
  - Pass-1 enc_proj runs mostly in fp8 e4m3 with MatmulPerfMode.DoubleRow
    (2 fp8 K-values per PE cell -> 2x bf16 throughput).  Quantization
    noise on the scores (~0.03 absolute) fails short-`lengths` batches
    whose softmax support is too small to average it out, so s < 512 is
    computed in bf16 instead: short batches become fully bf16-accurate
    and long batches keep enough support for the fp8 noise to wash out.
  - Both We variants are pre-scaled by 32 host-side (fp8: clears the
    e4m3 subnormal range; bf16: scale-free) so one 1/32 descale in the
    tanh activation covers every pass-1 tile.  fp8 K = 1024 maps to 4
    chunks of 256 with d = kc*256 + p*2 + i (pair axis = dim 1).
  - energy^T tiles are [e=128, s=1024] (two single-bank PSUM matmul
    groups per tile) so each tanh is one big ACT op with the per-batch
    bias (hid_proj + b) as a per-partition scalar.
  - hid_proj is computed transposed ([e-part, batch], bf16) as one
    64-matmul PSUM accumulation group -> bias needs no DRAM bounce.
  - The v-dot runs on the DVE as an all-bf16 scalar_tensor_tensor chain
    (2-byte SBUF operands unlock the DVE fast modes); the 128->1
    partition reduce is one 8-matmul PSUM group per [128, 1024] acc tile.
  - Scores live in [128, S/128] layout; masked softmax uses the static
    bound M = sum|v| >= max(score) instead of a max-reduce; the exp
    table is accurate even at exp(-25) (verified on hw); normalization
    by 1/sum is applied to the final context vector.
  - Pass 2 (context) re-loads enc in natural [s, d] layout as bf16, all
    tiles DMA'd ahead, and runs accumulating matmuls (attn column
    stationary, enc moving).
  - HBM/core: enc fp8 3.1MB + bf16 window 1MB + enc bf16 8.4MB + W 5.1MB.
"""

import numpy as np

B, S, D = 16, 2048, 1024
NCORES = 8
BL = B // NCORES
WSCALE = 32.0
CBF = 512         # s-prefix computed in bf16

_NC_CACHE = {}


def _build_program(bl, s, d, stage="all"):
    import concourse.bacc as bacc
    import concourse.bass as bass
    import concourse.mybir as mybir
    import concourse.tile as tile

    f32 = mybir.dt.float32
    bf16 = mybir.dt.bfloat16
    f8 = mybir.dt.float8e4
    i32 = mybir.dt.int32
    Tanh = mybir.ActivationFunctionType.Tanh
    Exp = mybir.ActivationFunctionType.Exp
    Identity = mybir.ActivationFunctionType.Identity
    Alu = mybir.AluOpType
    DR = mybir.MatmulPerfMode.DoubleRow

    kcn = d // 256       # fp8 DoubleRow K chunks (pairs on 128 partitions)
    dcn = d // 128       # bf16 K chunks
    ecn = d // 128       # e chunks
    sc2 = s // 128       # score chunks (s = f*128 + p layout)
    s8 = s - CBF         # fp8-region length
    DESCALE = 1.0 / WSCALE

    nc = bacc.Bacc()
    encT8_d = nc.declare_dram_parameter("encT8", [bl, 128, kcn, 2, s8], f8, isOutput=False)
    encTbf_d = nc.declare_dram_parameter("encTbf", [bl, 128, dcn, CBF], bf16, isOutput=False)
    encbf_d = nc.declare_dram_parameter("encbf", [bl, s, d], bf16, isOutput=False)
    wt8_d = nc.declare_dram_parameter("wt8", [128, kcn, 2, d], f8, isOutput=False)
    wtbf_d = nc.declare_dram_parameter("wtbf", [128, dcn, d], bf16, isOutput=False)
    whbf_d = nc.declare_dram_parameter("whbf", [128, dcn, d], bf16, isOutput=False)
    hidT_d = nc.declare_dram_parameter("hidT", [128, dcn, bl], bf16, isOutput=False)
    bcol_d = nc.declare_dram_parameter("bcol", [128, ecn], f32, isOutput=False)
    vcol_d = nc.declare_dram_parameter("vcol", [128, ecn], f32, isOutput=False)
    len_d = nc.declare_dram_parameter("len_i", [128, bl], i32, isOutput=False)
    if stage == "all":
        out_d = nc.declare_dram_parameter("ctx_out", [bl, d], f32, isOutput=True)
    else:
        out_d = nc.declare_dram_parameter("ctx_out", [bl, s], f32, isOutput=True)

    with tile.TileContext(nc) as tc:
        with (
            tc.tile_pool(name="consts", bufs=1) as consts,
            tc.tile_pool(name="etp", bufs=6) as etp,
            tc.tile_pool(name="enp", bufs=4) as enp,
            tc.tile_pool(name="p2p", bufs=6) as p2p,
            tc.tile_pool(name="sb1", bufs=1) as sb1,
            tc.tile_pool(name="psA", bufs=2, space="PSUM") as psA,
            tc.tile_pool(name="psS", bufs=2, space="PSUM") as psS,
            tc.tile_pool(name="psM", bufs=1, space="PSUM") as psM,
        ):
            # ---------------- startup DMAs (order = HBM priority) ----------
            # Sync queue feeds the pass-1 matmuls in consumption order
            # (fp8 s-half of batch 0 first -- it needs the fewest bytes);
            # Wh streams concurrently on the otherwise-idle Pool queue so
            # the bias chain is ready by the first tanh.
            hidT_sb = consts.tile([128, dcn, bl], bf16)
            nc.sync.dma_start(out=hidT_sb, in_=hidT_d[:, :, :])
            # Wh rides in pass-2 pool slots: read only by the hid matmul
            # group at startup, then the slots recycle into en2 tiles.
            wh_t = []
            for h in range(2):
                wh = p2p.tile([128, dcn // 2, d], bf16, tag="en2", name=f"wh{h}")
                nc.gpsimd.dma_start(
                    out=wh, in_=whbf_d[:, h * (dcn // 2):(h + 1) * (dcn // 2), :]
                )
                wh_t.append(wh)
            wt8_sb = consts.tile([128, kcn, 2, d], f8)
            nc.sync.dma_start(out=wt8_sb, in_=wt8_d[:, :, :, :])

            pre_et = {}

            def fetch_sh1(bb, split=False):
                e81 = etp.tile([128, kcn, 2, 1024], f8, tag="et", name=f"e81_{bb}")
                if split:
                    # per-kc DMAs so the first matmul group can start as
                    # soon as chunk 0 lands
                    for kc in range(kcn):
                        nc.sync.dma_start(
                            out=e81[:, kc, :, :],
                            in_=encT8_d[bb, :, kc, :, 1024 - CBF:s8],
                        )
                else:
                    nc.sync.dma_start(out=e81, in_=encT8_d[bb, :, :, :, 1024 - CBF:s8])
                pre_et[(bb, 1)] = (None, e81)

            def fetch_sh0(bb):
                ebf = etp.tile([128, dcn, CBF], bf16, tag="et", name=f"ebf{bb}")
                nc.sync.dma_start(out=ebf, in_=encTbf_d[bb, :, :, :])
                e80 = etp.tile([128, kcn, 2, 1024 - CBF], f8, tag="et", name=f"e80_{bb}")
                nc.sync.dma_start(out=e80, in_=encT8_d[bb, :, :, :, 0:1024 - CBF])
                pre_et[(bb, 0)] = (ebf, e80)

            fetch_sh1(0, split=True)
            fetch_sh1(1)
            fetch_sh0(0)
            wtbf_sb = consts.tile([128, dcn, d], bf16)
            nc.sync.dma_start(out=wtbf_sb, in_=wtbf_d[:, :, :])
            vcol_sb = consts.tile([128, ecn], f32)
            nc.gpsimd.dma_start(out=vcol_sb, in_=vcol_d[:, :])
            bcol_sb = consts.tile([128, ecn], f32)
            nc.gpsimd.dma_start(out=bcol_sb, in_=bcol_d[:, :])
            len_i_sb = consts.tile([128, bl], i32)
            nc.gpsimd.dma_start(out=len_i_sb, in_=len_d[:, :])

            # ---------------- small consts ----------------
            len_f_sb = consts.tile([128, bl], f32)
            nc.vector.tensor_copy(len_f_sb, len_i_sb)
            iotaT_i = consts.tile([128, sc2], i32)
            nc.gpsimd.iota(
                iotaT_i, pattern=[[128, sc2]], base=0, channel_multiplier=1
            )
            iotaT_f = consts.tile([128, sc2], f32)
            nc.vector.tensor_copy(iotaT_f, iotaT_i)
            ones_sb = consts.tile([128, 1], f32)
            nc.vector.memset(ones_sb, 1.0)
            ones_bf = consts.tile([128, 1], bf16)
            nc.vector.memset(ones_bf, 1.0)
            ones_row = consts.tile([1, 128], f32)
            nc.vector.memset(ones_row, 1.0)
            validT = []
            for b_ in range(bl):
                vv = consts.tile([128, sc2], f32, name=f"validT{b_}")
                nc.vector.tensor_scalar(
                    vv, iotaT_f, len_f_sb[:, b_:b_ + 1], None, op0=Alu.is_lt
                )
                validT.append(vv)

            # ---------------- hid_proj + b -> bias_all[:, ec, b] -----------
            # Transposed bf16 hid projection: one PSUM accumulation group
            # of dcn*ecn matmuls, each writing its own [128, bl] region of
            # a single bank -- no DRAM bounce needed to get the
            # [e-partition] layout the tanh bias wants.  The matmul group
            # is emitted MID-way through the first pass-1 tile (via mid_cb)
            # so its Wh-gated, ldweights-paced crawl overlaps real work;
            # the bias ACTs stay ahead of the first tanh on the ACT queue.
            ps_hbT = psS.tile([128, 16], f32, tag="s", name="ps_hbT")
            bias_all = consts.tile([128, ecn, bl], f32)

            def emit_hid_mms():
                for kc in range(dcn):
                    for ec in range(ecn):
                        nc.tensor.matmul(
                            ps_hbT[:, ec * bl:(ec + 1) * bl],
                            wh_t[kc // 4][:, kc % 4, ec * 128:(ec + 1) * 128],
                            hidT_sb[:, kc, :],
                            start=(kc == 0 and ec == 0),
                            stop=(kc == dcn - 1 and ec == ecn - 1),
                            skip_group_check=True,
                        )
                for ec in range(ecn):
                    nc.scalar.activation(
                        bias_all[:, ec, :],
                        ps_hbT[:, ec * bl:(ec + 1) * bl],
                        Identity,
                        bias=bcol_sb[:, ec:ec + 1],
                    )

            def emit_negM():
                # Upper bound M = sum|v| >= any score (|tanh|<=1), used
                # instead of the true max in softmax -- no max-reduce.
                vabs = consts.tile([128, 1], f32)
                nc.vector.reduce_sum(
                    out=vabs, in_=vcol_sb, axis=mybir.AxisListType.X,
                    apply_absolute_value=True,
                )
                psv = psS.tile([128, 16], f32, tag="s", name="psv")
                nc.tensor.matmul(
                    psv[0:1, 0:1], vabs, ones_sb[:, 0:1], start=True, stop=True
                )
                mtot = consts.tile([1, 1], f32)
                nc.vector.tensor_copy(mtot, psv[0:1, 0:1])
                psb = psS.tile([128, 16], f32, tag="s", name="psb")
                nc.tensor.matmul(
                    psb[:, 0:1], ones_row[:, :], mtot[:, :], start=True, stop=True
                )
                negM_bc = consts.tile([128, 1], f32)
                nc.scalar.mul(negM_bc, psb[:, 0:1], -1.0)
                return negM_bc

            scoresT = [
                consts.tile([128, sc2], f32, name=f"scoresT{b_}")
                for b_ in range(bl)
            ]
            attnT = [
                consts.tile([128, sc2], bf16, name=f"attnT{b_}")
                for b_ in range(bl)
            ]

            # ---------------- pass 1: scores ----------------
            def flush_pending(pending):
                # Deferred partition-reduce of the v-dot acc tile: one
                # 8-matmul PSUM group (each chunk c of acc column-sums into
                # its own column of a single bank) + one DVE copy out.
                acc_p, bb_p, sh_p = pending
                sps = psS.tile([128, 16], f32, tag="s")
                for c_ in range(8):
                    nc.tensor.matmul(
                        sps[:, c_:c_ + 1],
                        acc_p[:, c_ * 128:(c_ + 1) * 128],
                        ones_bf[:, 0:1],
                        start=(c_ == 0),
                        stop=(c_ == 7),
                        skip_group_check=True,
                    )
                nc.vector.tensor_copy(
                    scoresT[bb_p][:, sh_p * 8:(sh_p + 1) * 8], sps[:, 0:8]
                )

            def softmax(bb, negM_bc):
                # exp(score - M), mask + per-partition row-sum fused in one
                # DVE pass, then a 128->1 matmul for the total.
                attn_raw = sb1.tile([128, sc2], f32, tag="araw")
                nc.scalar.activation(
                    attn_raw, scoresT[bb], Exp, bias=negM_bc[:, 0:1]
                )
                attn_exp = sb1.tile([128, sc2], f32, tag="aexp")
                psums = sb1.tile([128, 1], f32, tag="psums")
                nc.vector.scalar_tensor_tensor(
                    attn_exp,
                    attn_raw,
                    1.0,
                    validT[bb],
                    op0=Alu.mult,
                    op1=Alu.mult,
                    accum_out=psums,
                )
                nc.vector.tensor_copy(attnT[bb], attn_exp)
                psm = psS.tile([128, 16], f32, tag="s")
                nc.tensor.matmul(
                    psm[0:1, 0:1], psums, ones_sb[:, 0:1], start=True, stop=True
                )
                rinv = sb1.tile([1, 1], f32, tag=f"rinv{bb}", name=f"rinv{bb}")
                nc.vector.reciprocal(rinv, psm[0:1, 0:1])
                if stage == "sm":
                    nc.gpsimd.dma_start(
                        out=out_d[bb, :].rearrange("(f p) -> p f", p=128),
                        in_=attn_exp,
                    )
                return rinv

            def p1_tile(bb, sh, pending, mid_cb=None):
                # One [128(e), 1024(s)] energy^T tile per ec: for sh=0 the
                # first bank is the bf16 window (s < CBF), the second bank
                # fp8; sh=1 is all fp8.  Both weight variants carry the x32
                # scale, so one tanh descale covers the whole tile.
                ebf, e8 = pre_et.pop((bb, sh))
                acc = enp.tile([128, 1024], bf16, tag="acc", bufs=3)
                for ec in range(ecn):
                    ps = psA.tile([128, 1024], f32, tag="proj")
                    if sh == 0:
                        for kc in range(dcn):
                            nc.tensor.matmul(
                                ps[:, 0:CBF],
                                wtbf_sb[:, kc, ec * 128:(ec + 1) * 128],
                                ebf[:, kc, :],
                                start=(kc == 0),
                                stop=(kc == dcn - 1),
                                skip_group_check=True,
                            )
                        f8_banks = [(CBF, 0)]
                    else:
                        f8_banks = [(0, 0), (512, 512)]
                    # fp8 DoubleRow: moving operands cap at 512 src elements
                    # = 256 out columns, so each 512-wide PSUM bank is one
                    # accumulation group of 4kc x 2 quarter matmuls.
                    for p0, s0 in f8_banks:
                        for kc in range(kcn):
                            for q in range(2):
                                nc.tensor.matmul(
                                    ps[:, p0 + q * 256:p0 + (q + 1) * 256],
                                    wt8_sb[:, kc, :, ec * 128:(ec + 1) * 128],
                                    e8[:, kc, :, s0 + q * 256:s0 + (q + 1) * 256],
                                    start=(kc == 0 and q == 0),
                                    stop=(kc == kcn - 1 and q == 1),
                                    perf_mode=DR,
                                    skip_group_check=True,
                                )
                    if ec == 2 and pending is not None:
                        flush_pending(pending)
                        pending = None
                    if mid_cb is not None:
                        # emitted after ec0's matmul groups, before the
                        # first tanh that consumes what it produces
                        mid_cb()
                        mid_cb = None
                    en = enp.tile([128, 1024], bf16, tag="en")
                    nc.scalar.activation(
                        en, ps, Tanh, bias=bias_all[:, ec, bb:bb + 1],
                        scale=DESCALE,
                    )
                    # v-dot on DVE: acc[p, s] += v[ec*128+p] * en[p, s];
                    # all-bf16 SBUF operands keep the DVE in its fast mode.
                    if ec == 0:
                        nc.vector.tensor_scalar_mul(acc, en, vcol_sb[:, 0:1])
                    else:
                        nc.vector.scalar_tensor_tensor(
                            acc,
                            en,
                            vcol_sb[:, ec:ec + 1],
                            acc,
                            op0=Alu.mult,
                            op1=Alu.add,
                        )
                if pending is not None:
                    flush_pending(pending)
                return (acc, bb, sh)

            en2_tiles = {}

            def fetch_en2(bb):
                for gi in range(4):
                    en2 = p2p.tile([128, 4, d], bf16, tag="en2")
                    nc.sync.dma_start(
                        out=en2,
                        in_=encbf_d[bb, gi * 512:(gi + 1) * 512, :].rearrange(
                            "(j p) e -> p j e", p=128
                        ),
                    )
                    en2_tiles[(bb, gi)] = en2

            def p2_group(bb, gi, cps):
                en2 = en2_tiles.pop((bb, gi))
                for j in range(4):
                    sci = gi * 4 + j
                    for h in range(2):
                        nc.tensor.matmul(
                            cps[:, h * 512:(h + 1) * 512],
                            attnT[bb][:, sci:sci + 1],
                            en2[:, j, h * 512:(h + 1) * 512],
                            start=(sci == 0),
                            stop=(sci == sc2 - 1),
                            skip_group_check=True,
                        )

            def p2_finish(bb, cps, rinv):
                ctx = sb1.tile([1, d], f32, tag=f"ctx{bb}", name=f"ctx{bb}")
                nc.vector.tensor_scalar_mul(ctx, cps, rinv[0:1, 0:1])
                nc.gpsimd.dma_start(out=out_d[bb:bb + 1, :], in_=ctx)

            # Emission order is PE execution order: pass-1 tiles ordered by
            # DMA-readiness (fp8-only halves need the fewest early bytes),
            # flushes deferred into the next tile's matmul stream, softmax
            # slotted where its tiny PE ops find their inputs long-ready,
            # pass-2 of b0 split around the last flush.  All pass-2 and
            # next-batch DMAs are issued well ahead of use.
            pending = p1_tile(0, 1, None, mid_cb=emit_hid_mms)
            pending = p1_tile(1, 1, pending)
            fetch_sh0(1)
            fetch_en2(0)
            pending = p1_tile(0, 0, pending)
            fetch_en2(1)
            pending = p1_tile(1, 0, pending)
            negM_bc = emit_negM()
            rinv0 = softmax(0, negM_bc)
            flush_pending(pending)
            rinv1 = softmax(1, negM_bc)
            if stage in ("p1", "sm"):
                if stage == "p1":
                    for bb in range(bl):
                        nc.gpsimd.dma_start(
                            out=out_d[bb, :].rearrange("(f p) -> p f", p=128),
                            in_=scoresT[bb],
                        )
            else:
                cps0 = psM.tile([1, d], f32, tag="m", name="cps0")
                for gi in range(4):
                    p2_group(0, gi, cps0)
                p2_finish(0, cps0, rinv0)
                cps1 = psM.tile([1, d], f32, tag="m", name="cps1")
                for gi in range(4):
                    p2_group(1, gi, cps1)
                p2_finish(1, cps1, rinv1)

    nc.compile()
    return nc


def _get_nc(bl=BL, s=S, d=D, stage="all"):
    key = (bl, s, d, stage)
    if key not in _NC_CACHE:
        _NC_CACHE[key] = _build_program(bl, s, d, stage)
    return _NC_CACHE[key]


def _to_e4m3(x, scale=1.0):
    import ml_dtypes

    return np.clip(
        np.asarray(x, dtype=np.float32) * scale, -240.0, 240.0
    ).astype(ml_dtypes.float8_e4m3)


def _pairchunk(xT):
    """[d, n] -> [p, kc, i, n] with d = kc*256 + p*2 + i."""
    d, n = xT.shape
    return np.ascontiguousarray(
        xT.reshape(d // 256, 128, 2, n).transpose(1, 0, 2, 3)
    )


def _chunk(xT):
    """[d, n] -> [p, kc, n] with d = kc*128 + p."""
    d, n = xT.shape
    return np.ascontiguousarray(xT.reshape(d // 128, 128, n).transpose(1, 0, 2))


def _make_in_maps(encoder_outputs, hidden, lengths, W, b, v):
    import ml_dtypes

    BF = ml_dtypes.bfloat16
    enc = np.asarray(encoder_outputs, dtype=np.float32)
    hid = np.asarray(hidden, dtype=np.float32)
    len_ = np.asarray(lengths, dtype=np.int32)
    Wn = np.asarray(W, dtype=np.float32)
    bn = np.asarray(b, dtype=np.float32)
    vn = np.asarray(v, dtype=np.float32)

    ecn = D // 128
    Wh, We = Wn[:, :D], Wn[:, D:]                        # [e, d] each
    wt8 = _pairchunk(_to_e4m3(We, WSCALE).T)
    wtbf = _chunk((We.T * WSCALE).astype(BF))
    whbf = _chunk(Wh.T.astype(BF))
    bcol = np.ascontiguousarray(bn.reshape(ecn, 128).T)  # [128, ecn]
    vcol = np.ascontiguousarray(vn.reshape(ecn, 128).T)
    enc8 = _to_e4m3(enc)                                 # [B, s, d]
    encbf = enc.astype(BF)
    in_maps = []
    for i in range(NCORES):
        sl = slice(BL * i, BL * (i + 1))
        in_maps.append(
            dict(
                encT8=np.stack(
                    [_pairchunk(enc8[j, CBF:].T) for j in range(sl.start, sl.stop)]
                ),
                encTbf=np.stack(
                    [_chunk(encbf[j, :CBF].T) for j in range(sl.start, sl.stop)]
                ),
                encbf=np.ascontiguousarray(encbf[sl]),
                wt8=wt8,
                wtbf=wtbf,
                whbf=whbf,
                hidT=_chunk(np.ascontiguousarray(hid[sl].T.astype(BF))),
                bcol=bcol,
                vcol=vcol,
                len_i=np.ascontiguousarray(
                    np.broadcast_to(len_[sl].reshape(1, BL), (128, BL)).copy()
                ),
            )
        )
    return in_maps


def run(inputs, trace=False, stage="all"):
    """Run on 8 NeuronCores; returns (output [B,1,D], BassKernelResults)."""
    from concourse.bass_utils import run_bass_kernel_spmd

    nc = _get_nc(stage=stage)
    in_maps = _make_in_maps(**inputs)
    r = run_bass_kernel_spmd(
        nc, in_maps, core_ids=list(range(NCORES)), trace=trace
    )
    out = np.concatenate(
        [np.asarray(r.results[i]["ctx_out"]) for i in range(NCORES)], axis=0
    )
    if stage != "all":
        return out, r
    return out[:, None, :].astype(np.float32), r


def kernel(encoder_outputs, hidden, lengths, W, b, v):
    out, _ = run(
        dict(
            encoder_outputs=encoder_outputs,
            hidden=hidden,
            lengths=lengths,
            W=W,
            b=b,
            v=v,
        )
    )
    return out
